# revision 20
# baseline (speedup 1.0000x reference)
"""Trainium2 Bass kernel for 8-head causal self-attention (b=2, s=4096, d=512, 8 heads x 64).

Sharding: 8 cores = 2 (batch) x 4 (head-pair). Core c handles batch c//4 and heads
(2*(c%4), 2*(c%4)+1). Each core computes a partial output projection over its two
heads' columns of W_O; the host sums the 4 partials per batch (tensor-parallel
all-reduce done on host at gather time).

Per-core algorithm ("everything transposed" layout, softmax over the partition axis):
  - host supplies x^T in bf16 (no on-device transposes of x)
  - K^T/Q^T/V^T projections (2 heads packed), V stationaries padded to 128 cols
    (FWL) with a fused ones-column for softmax sums
  - S^T[p,q] blocks (128p x 512q) via row-tiled matmuls (2 heads concurrent),
    diagonal blocks stream only the causal q-range; exp on ScalarE (scale=1/8)
    over double-buffered 2-bank PSUM groups; causal masking via GpSimd
    affine_select directly on the probability tiles
  - PV matmul accumulates z^T (+ row sums via the ones-column) in PSUM
  - epilogue: per-(q,head) reciprocal sums broadcast across partitions (GpSimd),
    z normalized in bf16 (DVE), single full-contraction output-projection matmul
    per 128-q chunk, bf16 partial written to DRAM (host accumulates in f32)
"""

import numpy as np
import ml_dtypes
from contextlib import ExitStack

import concourse.bass as bass
import concourse.mybir as mybir
import concourse.tile as tile
from concourse import bacc
from concourse.bass import ts, ds
from concourse.masks import make_identity

BF16 = mybir.dt.bfloat16
F32 = mybir.dt.float32

B, S, D, NH, DH = 2, 4096, 512, 8, 64
N_CORES = 8
QT = 512          # q tile (free dim of S^T blocks)
PC = 128          # p chunk (partition dim of S^T blocks)


def build_attention_core(s=S, d=D, dh=DH):
    nqt = s // QT
    n_kc = d // 128
    n_pct = s // PC
    nc = bacc.Bacc()
    xT_dram = nc.dram_tensor("xT", [d, s], BF16, kind="ExternalInput")
    wkT_dram = nc.dram_tensor("wkT", [d, 2 * dh], BF16, kind="ExternalInput")
    wqT_dram = nc.dram_tensor("wqT", [d, 2 * dh], BF16, kind="ExternalInput")
    wvT_dram = nc.dram_tensor("wvT", [d, 2 * dh], BF16, kind="ExternalInput")
    woT_dram = nc.dram_tensor("woT", [2 * dh, d], BF16, kind="ExternalInput")
    out_dram = nc.dram_tensor("out", [s, d], BF16, kind="ExternalOutput")

    with ExitStack() as ctx:
        tc = ctx.enter_context(tile.TileContext(nc))
        consts = ctx.enter_context(tc.tile_pool(name="consts", bufs=1))
        acts = ctx.enter_context(tc.tile_pool(name="acts", bufs=1))
        vstage = ctx.enter_context(tc.tile_pool(name="vstage", bufs=3))
        ptp = ctx.enter_context(tc.tile_pool(name="ptp", bufs=8))
        nrm = ctx.enter_context(tc.tile_pool(name="nrm", bufs=4))
        ost = ctx.enter_context(tc.tile_pool(name="ost", bufs=6))
        psp = ctx.enter_context(tc.tile_pool(name="psp", bufs=2, space="PSUM"))
        pzp = ctx.enter_context(tc.tile_pool(name="pzp", bufs=2, space="PSUM"))
        pmp = ctx.enter_context(tc.tile_pool(name="pmp", bufs=2, space="PSUM"))

        # ---- constants ----
        ident_bf = consts.tile([128, 128], BF16, tag="idb")
        make_identity(nc, ident_bf[:])
        # block one-hot [2, 128]: row h is 1 on cols [64h, 64h+64) — used to
        # broadcast per-head sums rows across the matching partition halves
        ones_row = consts.tile([1, 64], BF16, tag="ones")
        nc.gpsimd.memset(ones_row[:], 1.0)

        # ---- persistent activations / weights ----
        xT = acts.tile([128, n_kc, s], BF16, tag="xT")
        kT = acts.tile([128, s], BF16, tag="kT")   # rows 0-63 head A, 64-127 head B
        qT = acts.tile([128, s], BF16, tag="qT")
        # PV stationary per (p-chunk, head): cols 0-63 = V, col 64 = ones (sums)
        vt = acts.tile([128, n_pct, 2, 65], BF16, tag="vt")
        wk_sb = acts.tile([128, n_kc, 2 * dh], BF16, tag="wk")
        wq_sb = acts.tile([128, n_kc, 2 * dh], BF16, tag="wq")
        wv_sb = acts.tile([128, n_kc, 2 * dh], BF16, tag="wv")
        wo_sb = acts.tile([128, d], BF16, tag="wo")

        xT_src = xT_dram.rearrange("(kc p) s -> p kc s", p=128)
        # qt=0's x slice first so the first projections start ASAP
        for kc in range(n_kc):
            nc.sync.dma_start(xT[:, kc, ts(0, QT)], xT_src[:, kc, ts(0, QT)])
        nc.sync.dma_start(wk_sb[:], wkT_dram.rearrange("(kc p) h -> p kc h", p=128))
        nc.sync.dma_start(wq_sb[:], wqT_dram.rearrange("(kc p) h -> p kc h", p=128))
        nc.sync.dma_start(wv_sb[:], wvT_dram.rearrange("(kc p) h -> p kc h", p=128))
        nc.sync.dma_start(wo_sb[:], woT_dram[:])
        nc.gpsimd.memset(vt[:, :, :, dh : dh + 1], 1.0)

        # HAM warmup: keep the PE busy during the initial DMA so the clock
        # gate opens (1.2 -> 2.4 GHz) before the real matmuls arrive; also
        # pull the ACT exp-table load out of the first real chunk
        warm = pmp.tile([128, 128], BF16, tag="pmisc", name="warm")
        warm_sb = consts.tile([128, 128], BF16, tag="warm_sb")
        nc.tensor.transpose(warm[:], ident_bf[:], ident_bf[:])
        nc.scalar.activation(
            warm_sb[:], warm[:], mybir.ActivationFunctionType.Exp,
            bias=0.0, scale=0.0,
        )
        for i in range(90):
            nc.tensor.transpose(warm[:], ident_bf[:], ident_bf[:])

        def build(g):
            """Project K^T/Q^T/V^T and build V stationaries for p-slice [512g, 512g+512)."""
            if g > 0:
                for kc in range(n_kc):
                    nc.sync.dma_start(
                        xT[:, kc, ts(g, QT)], xT_src[:, kc, ts(g, QT)]
                    )
            vts = vstage.tile([128, QT], BF16, tag="vts", name=f"vts{g}")
            for w_sb, dst in ((wk_sb, kT[:, ts(g, QT)]), (wq_sb, qT[:, ts(g, QT)]), (wv_sb, vts[:])):
                pj = pmp.tile([128, 512], F32, tag="pmisc", name=f"pj{g}")
                for kc in range(n_kc):
                    nc.tensor.matmul(
                        pj[:, :], w_sb[:, kc, :], xT[:, kc, ts(g, QT)],
                        start=(kc == 0), stop=(kc == n_kc - 1),
                    )
                nc.vector.tensor_copy(dst, pj[:, :])
            for i in range(4):
                pc = 4 * g + i
                vtp = pmp.tile([128, 128], BF16, tag="pmisc", name=f"vtp{g}_{i}")
                nc.tensor.transpose(vtp[:], vts[:, ts(i, 128)], ident_bf[:])
                for h in range(2):
                    nc.vector.tensor_copy(vt[:, pc, h, 0:dh], vtp[:, ds(dh * h, dh)])

        def epi_rest(qt, zu, sums_sb):
            # deferred epilogue: 1/sums broadcast, z normalization, output projection
            zn = nrm.tile([128, QT], BF16, tag="zn", name=f"zn{qt}")
            pms = pmp.tile([128, 512], F32, tag="pmisc", name=f"pms{qt}")
            for h in range(2):  # broadcast head-h sums row across partitions 64h..64h+63
                nc.tensor.matmul(
                    pms[ds(dh * h, dh), :],
                    ones_row[:],
                    sums_sb[0:1, ts(h, QT)],
                    start=True, stop=True,
                    tile_position=(0, dh * h),
                )
            rsb = nrm.tile([128, QT], F32, tag="rsb", name=f"rsb{qt}")
            nc.vector.reciprocal_approx_fast(rsb[:], pms[:, :])
            nc.vector.tensor_mul(zn[:], zu[:], rsb[:])
            for qc in range(4):
                pout = pmp.tile([128, 512], F32, tag="pmisc", name=f"po{qt}_{qc}")
                nc.tensor.matmul(
                    pout[:, :], zn[:, ts(qc, 128)], wo_sb[:, :], start=True, stop=True
                )
                osb = ost.tile([128, d], BF16, tag="ob", name=f"ob{qt}_{qc}")
                nc.vector.tensor_copy(osb[:], pout[:, :])
                nc.sync.dma_start(out_dram[ds(QT * qt + 128 * qc, 128), :], osb[:])

        def emit_pv(zps, n_pc, pv_pc, pv_pt, q0):
            for h in ((0, 1) if pv_pc % 2 == 0 else (1, 0)):
                nc.tensor.matmul(
                    zps[h][:, ds(q0, QT - q0)],
                    vt[:, pv_pc, h, :],
                    pv_pt[:, ds(QT * h + q0, QT - q0)],
                    start=(pv_pc == 0), stop=(pv_pc == n_pc - 1),
                )

        def drain(qt, zps):
            # drain z^T + sums from PSUM (frees pz slots for the next q tile)
            zu = nrm.tile([128, QT], BF16, tag="zu", name=f"zu{qt}")
            sums_sb = nrm.tile([1, 1024], BF16, tag="sums", name=f"sums{qt}")
            for h in range(2):
                nc.vector.tensor_copy(zu[ds(dh * h, dh), :], zps[h][0:dh, :])
                nc.vector.tensor_copy(
                    sums_sb[:, ds(QT * h, QT)], zps[h][dh : dh + 1, :]
                )
            return (qt, zu, sums_sb)

        build(0)
        if nqt > 1:
            build(1)
        pending = None   # deferred epilogue (qt, zu, sums)
        pv_queue = []     # [(args, qt, is_last)] — PV trails S/exp by 2 chunks
        for qt in range(nqt):
            n_pc = (QT // PC) * (qt + 1)
            zps = [
                pzp.tile([dh + 1, QT], F32, tag="zps", name=f"zps{qt}_{h}")
                for h in range(2)
            ]
            for pc in range(n_pc):
                if pc == 2 and pending is not None:
                    epi_rest(*pending)
                    pending = None
                j = pc - (QT // PC) * qt
                q0 = PC * j if j > 0 else 0
                sps = psp.tile([128, 1024], F32, tag="sps", name=f"sps{qt}_{pc}")
                for h in range(2):
                    nc.tensor.matmul(
                        sps[:, ds(QT * h + q0, QT - q0)],
                        kT[ds(dh * h, dh), ts(pc, 128)],
                        qT[ds(dh * h, dh), ds(QT * qt + q0, QT - q0)],
                        start=True, stop=True,
                        tile_position=(dh * h, 0),
                    )
                pt_sb = ptp.tile([128, 1024], BF16, tag="pt", name=f"pt{qt}_{pc}")
                if q0 > 0:  # skip the never-read q-range on diagonal chunks
                    exp_in = sps[:].rearrange("p (h q) -> p h q", h=2)[:, :, q0:]
                    exp_out = pt_sb[:].rearrange("p (h q) -> p h q", h=2)[:, :, q0:]
                else:
                    exp_in, exp_out = sps[:], pt_sb[:]
                nc.scalar.activation(
                    exp_out, exp_in, mybir.ActivationFunctionType.Exp,
                    bias=0.0, scale=1.0 / np.sqrt(dh).item(),
                )
                if j >= 0:
                    # zero non-causal probs on the [128,128] triangle sub-block;
                    # columns left of it are trimmed out of the PV stream, so
                    # their (garbage-exp) contents are never read
                    for h in range(2):
                        blk = pt_sb[:, ds(QT * h + q0, PC)]
                        nc.gpsimd.affine_select(
                            out=blk, in_=blk,
                            compare_op=mybir.AluOpType.is_ge,
                            fill=0.0, base=0,
                            pattern=[[1, PC]], channel_multiplier=-1,
                        )
                if len(pv_queue) >= 3:
                    args, prev_qt, was_last = pv_queue.pop(0)
                    emit_pv(*args)
                    if was_last:
                        pending = drain(prev_qt, args[0])
                pv_queue.append(((zps, n_pc, pc, pt_sb, q0), qt, pc == n_pc - 1))
            if qt + 2 < nqt:
                build(qt + 2)
        for args, prev_qt, was_last in pv_queue:
            emit_pv(*args)
            if was_last:
                if pending is not None:
                    epi_rest(*pending)
                pending = drain(prev_qt, args[0])
        epi_rest(*pending)

    nc.finalize()
    return nc


_NC_CACHE = {}


def _get_nc(s=S):
    if s not in _NC_CACHE:
        _NC_CACHE[s] = build_attention_core(s=s)
    return _NC_CACHE[s]


def make_in_maps(x, W_K, W_Q, W_V, W_O):
    bf = ml_dtypes.bfloat16
    in_maps = []
    for c in range(N_CORES):
        b, hp = c // 4, c % 4
        hA, hB = 2 * hp, 2 * hp + 1
        wkT = np.concatenate([W_K[hA].T, W_K[hB].T], axis=1).astype(bf)  # [d, 128]
        wqT = np.concatenate([W_Q[hA].T, W_Q[hB].T], axis=1).astype(bf)
        wvT = np.concatenate([W_V[hA].T, W_V[hB].T], axis=1).astype(bf)
        woT = np.ascontiguousarray(W_O[:, DH * hA : DH * (hB + 1)].T).astype(bf)  # [128, d]
        in_maps.append(
            {
                "xT": np.ascontiguousarray(np.asarray(x[b]).T).astype(bf),  # [d, s]
                "wkT": np.ascontiguousarray(wkT),
                "wqT": np.ascontiguousarray(wqT),
                "wvT": np.ascontiguousarray(wvT),
                "woT": woT,
            }
        )
    return in_maps


def kernel(x, W_K, W_Q, W_V, W_O):
    from concourse.bass_utils import run_bass_kernel_spmd

    nc = _get_nc(S)
    in_maps = make_in_maps(x, W_K, W_Q, W_V, W_O)
    res = run_bass_kernel_spmd(nc, in_maps, core_ids=list(range(N_CORES)))
    out = np.zeros((B, S, D), dtype=np.float32)
    for c in range(N_CORES):
        out[c // 4] += np.asarray(res.results[c]["out"], dtype=np.float32)
    return out
